# revision 56
# baseline (speedup 1.0000x reference)
"""Multi-head attention (B=2, S=2048, D=1024, H=16) on 8 TRN2 NeuronCores.

Sharding: tensor-parallel over heads. Core c owns heads [2c, 2c+1]:
W_Q/W_K/W_V column slices [:, 128c:128(c+1)], W_O row slice
[128c:128(c+1), :]. Each core computes its partial output
x @ Wq_c ... @ Wo_c (full [B, S, D]); the host sums the 8 partials and
adds bo (output projection is linear, so row-parallel partial-sum is
exact).

Device kernel (per core, identical SPMD program, different weight data):
  - host passes x pre-transposed per batch: xT [B, D, S] (layout prep only)
  - QT/KT/VT = W_c^T @ xT  ([128 head-dims, S], 2 heads stacked on
    partitions), + per-dim biases; VT re-transposed on the PE into
    V-natural tiles [128 seq, dims] with a ones-column appended per head
    (lhsT = [V_h | 1] so PV's row 64 accumulates the softmax denominator).
  - scores are computed TRANSPOSED: S^T[k, q] = KT_h.T @ QT_h per
    128-k-tile; the two heads' score matmuls land on PE row groups 0:63
    and 64:127 and run CONCURRENTLY (tile_position row tiling).
  - exp is SPLIT between engines: most k-tiles on the scalar engine
    (table exp, [128, 2*QC] per instruction straight from PSUM), a
    fraction on the vector engine via the Schraudolph bit trick
    (int32(x*A+B) bitcast to fp32 ~= exp(x/8), ~2-3% per-element error
    that cancels in softmax normalization), rebalancing the exp load
    that otherwise paces the whole kernel.
  - the attention inner loop is software-pipelined: scores(t+1) is
    emitted before PV(t), so the PE never idles waiting for exp(t).
  - PV: ctx^T[dh|sum, q] += [V_h | 1].T @ E_h accumulated over k-tiles in
    PSUM; row 64 is the softmax denominator.
  - normalize: denominator rows broadcast across partitions with a K=1
    ones-matmul, fast reciprocal + multiply on DVE; head1 ctx rows
    DMA-shifted to partitions 64:127 for a single K=128 out-projection.
  - out-projection O[q, :] = ctx_norm^T.T @ Wo_c; the PSUM->SBUF moves
    alternate between scalar and vector engines (both ~0.7us/tile) to
    balance the two engines; DMA'd to DRAM.
  - batch 1's projection phase is chopped into small thunks interleaved
    between batch 0's attention k-tiles so the PE load stays smooth.

Matmul operands are bf16 (PE hides LDWEIGHTS in its background weight
buffer at ~216ns/matmul for N=512). Mask all-True (spec fill=ones).
"""

import numpy as np
from collections import deque

P = 128
DK = 64

B, S, D, H = 2, 2048, 1024, 16
N_CORES = 8
HPC = H // N_CORES
DHC = HPC * DK

# Schraudolph exp: int32(round(ss * A + B)) bitcast fp32 ~= exp(ss/8)
SCH_A = 0.125 * (2.0 ** 23) / np.log(2.0)
SCH_B = 127.0 * 2.0 ** 23 - 366392.0


def dve_tiles(nkt):
    """k-tiles whose exp runs on the vector engine (Schraudolph).
    Interleaved with scalar tiles so the two ss PSUM banks ping-pong
    between the two exp engines instead of serializing on one."""
    if nkt == 16:
        return {1, 4, 7, 10, 13}
    return {kt for kt in range(nkt) if kt % 3 == 1}


def build_nc(b=B, s=S, d=D, sc=512, qc=512):
    """Build the per-core Bass/Tile program. b/s/d parameterized so a
    scaled-down config can run in CoreSim."""
    import concourse.mybir as mybir
    from concourse import bacc
    import concourse.tile as tile
    from concourse.masks import make_identity

    f32 = mybir.dt.float32
    bf16 = mybir.dt.bfloat16
    i32 = mybir.dt.int32
    mult = mybir.AluOpType.mult
    add_op = mybir.AluOpType.add
    Exp = mybir.ActivationFunctionType.Exp

    SC, QC = sc, qc
    NSC = s // SC
    NQC = s // QC
    NKT = s // P
    DSUB = d // P
    QSUB = QC // P
    DVE_SET = dve_tiles(NKT)

    nc = bacc.Bacc("TRN2", target_bir_lowering=False, debug=False)

    # xT and w are pre-arranged on the host so each DMA reads contiguous
    # per-partition lines (8KB resp. 2KB) at full HBM bandwidth.
    xT_d = nc.dram_tensor("xT", [b, P, NSC, DSUB, SC], bf16,
                          kind="ExternalInput")
    w_d = {
        n: nc.dram_tensor(n, [P, DSUB, DHC], bf16, kind="ExternalInput")
        for n in ("wq", "wk", "wv")
    }
    bias_d = {
        n: nc.dram_tensor(n, [DHC], f32, kind="ExternalInput")
        for n in ("bq", "bk", "bv")
    }
    wo_d = nc.dram_tensor("wo", [DHC, d], bf16, kind="ExternalInput")
    out_d = nc.dram_tensor("out", [b, s, d], bf16, kind="ExternalOutput")

    with tile.TileContext(nc) as tc:
        with (
            tc.tile_pool(name="consts", bufs=1) as consts,
            tc.tile_pool(name="qkv", bufs=2) as qkv_pool,
            tc.tile_pool(name="xt", bufs=6) as xt_pool,
            tc.tile_pool(name="vt", bufs=2) as vt_pool,
            tc.tile_pool(name="e", bufs=6) as e_pool,
            tc.tile_pool(name="ei", bufs=4) as ei_pool,
            tc.tile_pool(name="norm", bufs=2) as norm_pool,
            tc.tile_pool(name="osb", bufs=2) as o_pool,
            tc.tile_pool(name="ps_s", bufs=2, space="PSUM") as ps_scores,
            tc.tile_pool(name="ps_c", bufs=1, space="PSUM") as ps_ctx,
            tc.tile_pool(name="ps_u", bufs=2, space="PSUM") as ps_util,
        ):
            w_sb = {}
            b_sb = {}
            wo_sb = consts.tile([P, d], bf16, tag="wo", name="wo")
            ones_sb = consts.tile([DK + 1, DK], bf16, tag="ones", name="ones")
            ident = consts.tile([P, P], bf16, tag="ident", name="ident")

            def load_consts():
                # gpsimd DMA queue, interleaved so wq (gating the first
                # projection matmul) lands first; all overlap the xT chunk-0
                # DMA on the sync queue
                for n in ("wq", "wk", "wv"):
                    t = consts.tile([P, DSUB, DHC], bf16, tag=n, name=n)
                    nc.gpsimd.dma_start(t, w_d[n].ap())
                    w_sb[n] = t
                    bt = consts.tile([P, 1], f32, tag="b" + n[1],
                                     name="b" + n[1])
                    nc.gpsimd.dma_start(bt, bias_d["b" + n[1]].ap()[:, None])
                    b_sb[n] = bt
                make_identity(nc, ident)

            def load_consts_late():
                nc.gpsimd.dma_start(wo_sb, wo_d.ap())
                nc.vector.memset(ones_sb[DK : DK + 1, :], 1.0)

            def phase1_alloc(bi):
                st = {
                    "b": bi,
                    "QT": qkv_pool.tile([P, s], bf16, tag="qt", name="qt"),
                    "KT": qkv_pool.tile([P, s], bf16, tag="kt", name="kt"),
                    "V": qkv_pool.tile([P, NKT, 2, DK + 2], bf16, tag="v",
                                       name="v"),
                    "xts": {},
                    "nproj": {},
                }
                nc.vector.memset(st["V"][:, :, 0, DK : DK + 1], 1.0)
                nc.vector.memset(st["V"][:, :, 1, DK : DK + 1], 1.0)
                return st

            def load_xt(st, sci):
                bi = st["b"]
                xt = xt_pool.tile([P, DSUB, SC], bf16, tag="xt", name="xt")
                # alternate DMA issue queues so chunk loads run in parallel;
                # batch 1 loads issue from the (idle-during-attn) gpsimd queue
                if bi == 1:
                    eng = nc.gpsimd
                else:
                    eng = nc.sync if sci % 2 == 0 else nc.scalar
                eng.dma_start(xt, xT_d.ap()[bi][:, sci])
                st["xts"][sci] = xt

            def proj_one(st, sci, n):
                """One projection (wq|wk|wv) for s-chunk sci."""
                ssl = slice(sci * SC, (sci + 1) * SC)
                if sci not in st["xts"]:
                    load_xt(st, sci)
                xt = st["xts"][sci]
                st["nproj"][sci] = st["nproj"].get(sci, 0) + 1
                dest = {"wq": st["QT"], "wk": st["KT"], "wv": None}[n]
                ps = ps_util.tile([P, SC], f32, tag="util", name="util")
                for o in range(DSUB):
                    nc.tensor.matmul(
                        ps, w_sb[n][:, o], xt[:, o],
                        start=(o == 0), stop=(o == DSUB - 1),
                    )
                badd = b_sb[n][:, 0:1].to_broadcast((P, SC))
                if st["nproj"][sci] == 3:
                    st["xts"].pop(sci)
                if dest is not None:
                    nc.vector.tensor_tensor(dest[:, ssl], ps, badd, add_op)
                else:
                    vt = vt_pool.tile([P, SC], bf16, tag="vt", name="vt")
                    nc.vector.tensor_tensor(vt, ps, badd, add_op)
                    for j in range(SC // P):
                        kti = (sci * SC) // P + j
                        ps_t = ps_util.tile([P, P], bf16, tag="util",
                                            name="util")
                        nc.tensor.transpose(ps_t, vt[:, j * P : (j + 1) * P],
                                            ident)
                        nc.vector.tensor_copy(
                            out=st["V"][:, kti, 0, 0:DK], in_=ps_t[:, 0:DK]
                        )
                        nc.vector.tensor_copy(
                            out=st["V"][:, kti, 1, 0:DK],
                            in_=ps_t[:, DK : 2 * DK],
                        )

            def phase1_chunk(st, sci):
                for n in ("wq", "wk", "wv"):
                    proj_one(st, sci, n)

            def phase1_thunks(st):
                """Batch-1 K/V projections as a deque of small emissions.
                Q projections (except chunk 0) are deferred into batch 1's
                own attention window, which has PE slack."""
                work = deque()
                work.append(lambda: load_xt(st, 0))
                for sci in range(NSC):
                    work.append(lambda sci=sci: proj_one(st, sci, "wk"))
                    if sci + 1 < NSC:
                        work.append(lambda sci=sci: load_xt(st, sci + 1))
                    work.append(lambda sci=sci: proj_one(st, sci, "wv"))
                work.append(lambda: proj_one(st, 0, "wq"))
                return work

            def attn_begin(st, qci):
                return {
                    "st": st, "qci": qci,
                    "ctx0": ps_ctx.tile([DK + 1, QC], f32, tag="h0",
                                        name="h0"),
                    "ctx1": ps_ctx.tile([DK + 1, QC], f32, tag="h1",
                                        name="h1"),
                    "pend": deque(),
                }

            def _scores(ac, kt):
                st, qci = ac["st"], ac["qci"]
                QT, KT = st["QT"], st["KT"]
                qsl = slice(qci * QC, (qci + 1) * QC)
                ksl = slice(kt * P, (kt + 1) * P)
                ss = ps_scores.tile([P, 2 * QC], f32, tag="ss", name="ss")
                nc.tensor.matmul(
                    ss[:, 0:QC], KT[0:DK, ksl], QT[0:DK, qsl],
                    start=True, stop=True,
                )
                nc.tensor.matmul(
                    ss[:, QC : 2 * QC], KT[DK : 2 * DK, ksl],
                    QT[DK : 2 * DK, qsl], start=True, stop=True,
                )
                return ss

            def _expand(kt, ss):
                E = e_pool.tile([P, 2 * QC], bf16, tag="e", name="e")
                if kt in DVE_SET:
                    Ei = ei_pool.tile([P, 2 * QC], i32, tag="ei", name="ei")
                    nc.vector.tensor_scalar(Ei, ss, SCH_A, SCH_B,
                                            mult, add_op)
                    nc.vector.tensor_copy(out=E, in_=Ei.bitcast(f32))
                elif kt == NKT - 1:
                    # split the chunk's last exp per head so PV(h0) and the
                    # t0 staging start earlier, shortening the ctx handoff
                    nc.scalar.activation(E[:, 0:QC], ss[:, 0:QC], Exp,
                                         scale=1.0 / np.sqrt(DK))
                    nc.scalar.activation(E[:, QC : 2 * QC],
                                         ss[:, QC : 2 * QC], Exp,
                                         scale=1.0 / np.sqrt(DK))
                else:
                    nc.scalar.activation(E, ss, Exp, scale=1.0 / np.sqrt(DK))
                return E

            def _pv(ac, kt, E):
                V = ac["st"]["V"]
                nc.tensor.matmul(
                    ac["ctx0"], V[:, kt, 0, 0 : DK + 1], E[:, 0:QC],
                    start=(kt == 0), stop=(kt == NKT - 1),
                )
                nc.tensor.matmul(
                    ac["ctx1"], V[:, kt, 1, 0 : DK + 1], E[:, QC : 2 * QC],
                    start=(kt == 0), stop=(kt == NKT - 1),
                )

            def attn_steps(ac, kts, work=None):
                # software pipeline: PV lags two k-tiles so even the slower
                # DVE exp chain is done before PV needs E.
                for kt in kts:
                    ss = _scores(ac, kt)
                    E = _expand(kt, ss)
                    ac["pend"].append((kt, E))
                    if len(ac["pend"]) > 3:
                        _pv(ac, *ac["pend"].popleft())
                    # thunk slots avoid the DVE-exp k-tiles (kt%3==1) so the
                    # thunks' DVE work never lands on a Schraudolph tile
                    if work and kt % 3 == 0 and kt < NKT - 1 and work:
                        work.popleft()()

            def attn_finish(ac):
                while ac["pend"]:
                    _pv(ac, *ac["pend"].popleft())
                # stage ctx (incl. denominator row DK) to SBUF on BOTH
                # engines in parallel; releases the ctx PSUM banks.
                t0 = norm_pool.tile([DK + 1, QC], bf16, tag="t0", name="t0")
                nc.scalar.copy(out=t0, in_=ac["ctx0"])
                t1 = norm_pool.tile([DK + 1, QC], bf16, tag="t1", name="t1")
                nc.vector.tensor_copy(out=t1, in_=ac["ctx1"])
                return t0, t1

            def attn_core(st, qci, work=None):
                ac = attn_begin(st, qci)
                attn_steps(ac, range(NKT), work)
                return attn_finish(ac)

            def attn_outproj(st, qci, tt):
                bi = st["b"]
                t0, t1 = tt
                rbp = ps_util.tile([P, QC], f32, tag="util", name="util")
                nc.tensor.matmul(
                    rbp[0:DK, :], ones_sb[DK : DK + 1, :], t0[DK : DK + 1, :],
                    start=True, stop=True,
                )
                nc.tensor.matmul(
                    rbp[DK : 2 * DK, :], ones_sb[DK : DK + 1, :],
                    t1[DK : DK + 1, :], start=True, stop=True,
                )
                rc = norm_pool.tile([P, QC], f32, tag="rc", name="rc")
                nc.vector.reciprocal_approx_fast(out=rc, in_=rbp)
                tmp = norm_pool.tile([P, QC], bf16, tag="tmp", name="tmp")
                nc.sync.dma_start(tmp[DK : 2 * DK, :], t1[0:DK, :])
                cn = norm_pool.tile([P, QC], bf16, tag="cn", name="cn")
                nc.vector.tensor_tensor(cn[0:DK], t0[0:DK], rc[0:DK], mult)
                nc.vector.tensor_tensor(
                    cn[DK : 2 * DK], tmp[DK : 2 * DK], rc[DK : 2 * DK], mult
                )
                OH = min(512, d)
                for qs in range(QSUB):
                    osb = o_pool.tile([P, d], bf16, tag="osb", name="osb")
                    for h in range(d // OH):
                        ps_o = ps_util.tile([P, OH], f32, tag="util",
                                            name="util")
                        nc.tensor.matmul(
                            ps_o, cn[:, qs * P : (qs + 1) * P],
                            wo_sb[:, h * OH : (h + 1) * OH],
                            start=True, stop=True,
                        )
                        # balance the PSUM->SBUF moves across engines
                        if (qs + h) % 2 == 0:
                            nc.vector.tensor_copy(
                                out=osb[:, h * OH : (h + 1) * OH], in_=ps_o
                            )
                        else:
                            nc.scalar.copy(
                                out=osb[:, h * OH : (h + 1) * OH], in_=ps_o
                            )
                    row0 = qci * QC + qs * P
                    # last chunk's stores on sync only, so the gpsimd queue
                    # can drain during (not after) the final out-projection
                    last = qci == NQC - 1
                    oeng = nc.sync if (last or qs % 2 == 0) else nc.gpsimd
                    oeng.dma_start(out_d.ap()[bi, row0 : row0 + P, :], osb)

            st0 = phase1_alloc(0)
            load_xt(st0, 0)
            load_consts()
            load_consts_late()
            SCP = SC // P  # k-tiles made available per projection chunk
            early = QC <= SC  # q-chunk 0 only needs QT from s-chunk 0
            phase1_chunk(st0, 0)
            ac0 = attn_begin(st0, 0) if early else None
            for sci in range(1, NSC):
                if sci + 1 < NSC:
                    load_xt(st0, sci + 1)
                phase1_chunk(st0, sci)
                if early:
                    # q0 attention for the k-tiles unlocked by chunk sci-1:
                    # its exp work hides inside the projection PE window
                    attn_steps(ac0, range((sci - 1) * SCP, sci * SCP))
            if b > 1:
                st1 = phase1_alloc(1)
                work = phase1_thunks(st1)
                cns = {}
                if early:
                    attn_steps(ac0, range((NSC - 1) * SCP, NKT), work)
                    cns[(0, 0)] = attn_finish(ac0)
                else:
                    cns[(0, 0)] = attn_core(st0, 0, work)
                for qci in range(1, NQC):
                    cns[(0, qci)] = attn_core(st0, qci, work)
                    attn_outproj(st0, qci - 1, cns.pop((0, qci - 1)))
                while work:
                    work.popleft()()
                attn_outproj(st0, NQC - 1, cns.pop((0, NQC - 1)))
                # batch 1 attention; its remaining Q projections ride along
                qwork = deque(
                    (lambda qq=q: proj_one(st1, qq, "wq"))
                    for q in range(1, NQC)
                )
                cns[(1, 0)] = attn_core(st1, 0, qwork)
                for qci in range(1, NQC):
                    cns[(1, qci)] = attn_core(st1, qci, qwork)
                    attn_outproj(st1, qci - 1, cns.pop((1, qci - 1)))
                attn_outproj(st1, NQC - 1, cns.pop((1, NQC - 1)))
            else:
                work = deque()
                if early:
                    attn_steps(ac0, range((NSC - 1) * SCP, NKT))
                    cn_prev = attn_finish(ac0)
                else:
                    cn_prev = attn_core(st0, 0)
                for qci in range(1, NQC):
                    cn = attn_core(st0, qci)
                    attn_outproj(st0, qci - 1, cn_prev)
                    cn_prev = cn
                attn_outproj(st0, NQC - 1, cn_prev)

    nc.compile()
    return nc


_NC_CACHE = {}


def _get_nc():
    if "nc" not in _NC_CACHE:
        _NC_CACHE["nc"] = build_nc()
    return _NC_CACHE["nc"]


def prep_xT(x_f32, sc):
    """[B, S, D] fp32 -> [B, P, NSC, DSUB, SC] bf16, contiguous DMA lines."""
    import ml_dtypes

    b, s, d = x_f32.shape
    nsc, dsub = s // sc, d // P
    xr = x_f32.reshape(b, nsc, sc, dsub, P).transpose(0, 4, 1, 3, 2)
    return np.ascontiguousarray(xr).astype(ml_dtypes.bfloat16)


def prep_w(w_slice):
    """[d, DHC] -> [P, DSUB, DHC] matching the on-device weight layout."""
    dd = w_slice.shape[0]
    return np.ascontiguousarray(
        w_slice.reshape(dd // P, P, w_slice.shape[1]).transpose(1, 0, 2)
    )


def make_in_maps(inputs):
    import ml_dtypes

    bf16 = ml_dtypes.bfloat16
    x = np.ascontiguousarray(np.asarray(inputs["x"], dtype=np.float32))
    xT = prep_xT(x, 512)
    Wq = np.asarray(inputs["Wq"], dtype=np.float32).astype(bf16)
    Wk = np.asarray(inputs["Wk"], dtype=np.float32).astype(bf16)
    Wv = np.asarray(inputs["Wv"], dtype=np.float32).astype(bf16)
    Wo = np.asarray(inputs["Wo"], dtype=np.float32).astype(bf16)
    bq = np.asarray(inputs["bq"], dtype=np.float32)
    bk = np.asarray(inputs["bk"], dtype=np.float32)
    bv = np.asarray(inputs["bv"], dtype=np.float32)
    in_maps = []
    for c in range(N_CORES):
        sl = slice(c * DHC, (c + 1) * DHC)
        in_maps.append(
            {
                "xT": xT,
                "wq": prep_w(Wq[:, sl]),
                "wk": prep_w(Wk[:, sl]),
                "wv": prep_w(Wv[:, sl]),
                "bq": np.ascontiguousarray(bq[sl]),
                "bk": np.ascontiguousarray(bk[sl]),
                "bv": np.ascontiguousarray(bv[sl]),
                "wo": np.ascontiguousarray(Wo[sl, :]),
            }
        )
    return in_maps


def run(inputs, trace=False):
    """Run on 8 NeuronCores; returns (output, BassKernelResults)."""
    from concourse.bass_utils import run_bass_kernel_spmd

    nc = _get_nc()
    res = run_bass_kernel_spmd(
        nc, make_in_maps(inputs), core_ids=list(range(N_CORES)), trace=trace
    )
    bo = np.asarray(inputs["bo"], dtype=np.float32)
    out = np.zeros((B, S, D), dtype=np.float32)
    for rmap in res.results:
        out += np.asarray(rmap["out"]).astype(np.float32)
    out += bo[None, None, :]
    return out, res


def kernel(**inputs):
    out, _ = run(inputs, trace=False)
    return out
